# revision 17
# baseline (speedup 1.0000x reference)
"""GQA cross-attention block on 8 trn2 NeuronCores (v2).

Sharding: tensor-parallel over heads. Core c owns KV group g=c (64 dims of
K/V) and its 4 query heads (256 q channels). Each core computes its heads'
attention plus its slice of the o-projection (rows c*256:(c+1)*256 of Wo),
producing a full-shape partial output; the host sums the 8 partials and
adds bo. No device collectives needed.

v2 schedule (vs v1): K|V projection fused into one M=128 matmul chain;
scores row-tiled via tile_position so two heads' K=64 matmuls share the PE
array; exp runs on [128,1024] two-bank PSUM spans (half the ACT instruction
overhead); softmax normalization uses reciprocal_approx_fast + gpsimd
partition_broadcast instead of the 1-partition DVE reciprocal + ones-matmul
chain; phases of consecutive batches are emitted so the Tile scheduler can
fill PE gaps (projection/o-proj matmuls interleave with the ACT-bound
attention loop, keeping the PE HAM clock-gate at 8/8).

Device layouts (host prepares):
  xT, encT: [B, HIDDEN, S] bf16, wq [HID,256], wkv [HID,128] (k|v cols),
  wo [256, HID] bf16, biases as fp32 columns.
"""

import numpy as np
import ml_dtypes

import concourse.bass as bass
from concourse import bacc
import concourse.mybir as mybir
import concourse.tile as tile
from concourse.bass_utils import run_bass_kernel_spmd
from concourse.masks import make_identity

BF16 = ml_dtypes.bfloat16
F32 = mybir.dt.float32
BF = mybir.dt.bfloat16

B = 2
S = 2048
HID = 2048
D = 64          # head dim
RQ = 4          # query heads per core (per kv group)
CH = RQ * D     # 256 q channels per core
NCORES = 8
NH = HID // 128  # 16 hidden chunks
NST = S // 512   # 4 s-tiles of 512
NKC = S // 128   # 16 key chunks of 128
SCALE = 1.0 / np.sqrt(D)

ID = mybir.ActivationFunctionType.Identity
EXP = mybir.ActivationFunctionType.Exp


def _build_nc() -> bass.Bass:
    nc = bacc.Bacc()

    xT = nc.dram_tensor("xT", [B, HID, S], BF, kind="ExternalInput")
    encT = nc.dram_tensor("encT", [B, HID, S], BF, kind="ExternalInput")
    wq = nc.dram_tensor("wq", [HID, CH], BF, kind="ExternalInput")
    wkv = nc.dram_tensor("wkv", [HID, 128], BF, kind="ExternalInput")
    wo = nc.dram_tensor("wo", [CH, HID], BF, kind="ExternalInput")
    bq = nc.dram_tensor("bq", [CH, 1], F32, kind="ExternalInput")
    bk = nc.dram_tensor("bk", [D, 1], F32, kind="ExternalInput")
    bv = nc.dram_tensor("bv", [D, 1], F32, kind="ExternalInput")
    out = nc.dram_tensor("out", [B, S, HID], BF, kind="ExternalOutput")

    with tile.TileContext(nc) as tc:
        with (
            tc.tile_pool(name="wpool", bufs=1) as wpool,
            tc.tile_pool(name="xs", bufs=20) as xs_pool,
            tc.tile_pool(name="es", bufs=20) as es_pool,
            tc.tile_pool(name="acts", bufs=2) as acts,
            tc.tile_pool(name="vaug", bufs=2) as vaug_pool,
            tc.tile_pool(name="epool", bufs=4) as epool,
            tc.tile_pool(name="small", bufs=2) as small,
            tc.tile_pool(name="osb", bufs=4) as osb_pool,
            tc.tile_pool(name="psum", bufs=2, space="PSUM") as ps,
        ):
            # ---- resident weights ----
            wq_t = []
            wkv_t = []
            for h in range(NH):
                wqh = wpool.tile([128, CH], BF, name=f"wq{h}")
                nc.sync.dma_start(out=wqh[:], in_=wq[h * 128:(h + 1) * 128, :])
                wq_t.append(wqh)
                wkvh = wpool.tile([128, 128], BF, name=f"wkv{h}")
                nc.sync.dma_start(out=wkvh[:], in_=wkv[h * 128:(h + 1) * 128, :])
                wkv_t.append(wkvh)
            wo_t = []
            for cchunk in range(2):
                woc = wpool.tile([128, HID], BF, name=f"wo{cchunk}")
                nc.sync.dma_start(out=woc[:], in_=wo[cchunk * 128:(cchunk + 1) * 128, :])
                wo_t.append(woc)
            bq_t = []
            for cchunk in range(2):
                bqc = wpool.tile([128, 1], F32, name=f"bq{cchunk}")
                nc.sync.dma_start(out=bqc[:], in_=bq[cchunk * 128:(cchunk + 1) * 128, :])
                bq_t.append(bqc)
            bk_t = wpool.tile([D, 1], F32, name="bk_t")
            nc.sync.dma_start(out=bk_t[:], in_=bk[:, :])
            bv_t = wpool.tile([D, 1], F32, name="bv_t")
            nc.sync.dma_start(out=bv_t[:], in_=bv[:, :])

            ident = wpool.tile([128, 128], BF, name="ident")
            make_identity(nc, ident[:])

            # per-batch persistent activation tiles (bufs=2 rotation)
            kdup_b = []
            vt_b = []
            qp_b = []
            otu_b = []
            vaug_b = []
            for b in range(B):
                kdup_b.append(acts.tile([128, S], BF, tag="kdup", name=f"kdup{b}"))
                vt_b.append(acts.tile([D, S], BF, tag="vt", name=f"vt{b}"))
                qp_b.append([
                    acts.tile([128, S], BF, tag=f"qp{p}", name=f"qp{p}_{b}")
                    for p in range(2)
                ])
                otu_b.append([
                    acts.tile([128, S], BF, tag=f"otu{p}", name=f"otu{p}_{b}")
                    for p in range(2)
                ])
                vaug_b.append([
                    vaug_pool.tile([128, 128], BF, tag=f"va{kc}", name=f"va{b}{kc}")
                    for kc in range(NKC)
                ])

            def emit_proj(b):
                kdup = kdup_b[b]
                vt = vt_b[b]
                # KV projection (k rows 0:64, v rows 64:128 of psum); st in
                # pairs with the stationary wkv chunk reused for both s-tiles
                for stp in range(NST // 2):
                    st0, st1 = 2 * stp, 2 * stp + 1
                    kv0 = ps.tile([128, 512], F32, tag="pproj", name=f"kvps{b}{st0}")
                    kv1 = ps.tile([128, 512], F32, tag="pproj", name=f"kvps{b}{st1}")
                    psl = slice(st0 * 512, (st1 + 1) * 512)
                    for h in range(NH):
                        et = es_pool.tile([128, 1024], BF, tag="es",
                                          name=f"es{b}{stp}{h}")
                        # batch 0 enc rides the ramp-idle scalar engine;
                        # batch 1 issues during attn(0) when ACT is saturated
                        (nc.scalar if b == 0 else nc.gpsimd).dma_start(
                            out=et[:], in_=encT[b, h * 128:(h + 1) * 128, psl])
                        nc.tensor.matmul(
                            kv0[:], wkv_t[h][:], et[:, 0:512],
                            start=(h == 0), stop=(h == NH - 1))
                        nc.tensor.matmul(
                            kv1[:], wkv_t[h][:], et[:, 512:1024],
                            start=(h == 0), stop=(h == NH - 1))
                    for st, kvps in ((st0, kv0), (st1, kv1)):
                        ssl = slice(st * 512, (st + 1) * 512)
                        nc.vector.tensor_scalar_add(
                            kdup[0:D, ssl], kvps[0:D, :], bk_t[:])
                        nc.scalar.activation(
                            kdup[D:128, ssl], kvps[0:D, :], ID, bias=bk_t[:])
                        nc.scalar.activation(
                            vt[0:D, ssl], kvps[D:128, :], ID, bias=bv_t[:])
                # v transposes -> v_aug chunks [128 kpos, 65] with ones col
                for kc in range(NKC):
                    vtp = ps.tile([128, D], BF, tag="pproj", name=f"vtp{b}{kc}")
                    nc.tensor.transpose(
                        vtp[:], vt[:, kc * 128:(kc + 1) * 128], ident[0:D, 0:D])
                    va = vaug_b[b][kc]
                    nc.gpsimd.memset(va[:, D:D + 1], 1.0)
                    nc.gpsimd.memset(va[:, D + 1:128], 0.0)
                    nc.vector.tensor_copy(va[:, 0:D], vtp[:])
                # Q projection -> head-pair tiles (heads 2p, 2p+1 stacked).
                # st processed in pairs with the stationary weight reused for
                # both s-tiles (halves LDWEIGHTS pressure); lo/hi in separate
                # sweeps so only 2 PSUM banks are live.
                for stp in range(NST // 2):
                    st0, st1 = 2 * stp, 2 * stp + 1
                    ssl0 = slice(st0 * 512, (st0 + 1) * 512)
                    ssl1 = slice(st1 * 512, (st1 + 1) * 512)
                    psl = slice(st0 * 512, (st1 + 1) * 512)
                    xts = []
                    for h in range(NH):
                        xt = xs_pool.tile([128, 1024], BF, tag="xs",
                                          name=f"xs{b}{stp}{h}")
                        nc.gpsimd.dma_start(
                            out=xt[:], in_=xT[b, h * 128:(h + 1) * 128, psl])
                        xts.append(xt)
                    for half, bias_t, qdst in (
                        (slice(0, 128), bq_t[0], qp_b[b][0]),
                        (slice(128, 256), bq_t[1], qp_b[b][1]),
                    ):
                        q0 = ps.tile([128, 512], F32, tag="pproj",
                                     name=f"q{half.start}{b}{st0}")
                        q1 = ps.tile([128, 512], F32, tag="pproj",
                                     name=f"q{half.start}{b}{st1}")
                        for h in range(NH):
                            nc.tensor.matmul(
                                q0[:], wq_t[h][:, half], xts[h][:, 0:512],
                                start=(h == 0), stop=(h == NH - 1))
                            nc.tensor.matmul(
                                q1[:], wq_t[h][:, half], xts[h][:, 512:1024],
                                start=(h == 0), stop=(h == NH - 1))
                        nc.vector.tensor_scalar_add(qdst[:, ssl0], q0[:], bias_t[:])
                        nc.vector.tensor_scalar_add(qdst[:, ssl1], q1[:], bias_t[:])

            def emit_attn(b, inline_oproj=False):
                kdup = kdup_b[b]
                for qc in range(NST):
                    qsl = slice(qc * 512, (qc + 1) * 512)
                    for pair in range(2):
                        qp = qp_b[b][pair]
                        otu = otu_b[b][pair]
                        avE = ps.tile([128, 512], F32, tag="av",
                                      name=f"avE{b}{pair}{qc}")
                        avO = ps.tile([128, 512], F32, tag="av",
                                      name=f"avO{b}{pair}{qc}")
                        for kc in range(NKC):
                            ksl = slice(kc * 128, (kc + 1) * 128)
                            sct = ps.tile([128, 1024], F32, tag="sct",
                                          name=f"sct{b}{pair}{qc}{kc}")
                            # row-tiled pair: head 2p on PE rows 0:64,
                            # head 2p+1 on rows 64:128 (concurrent)
                            nc.tensor.matmul(
                                sct[:, 0:512], kdup[0:D, ksl], qp[0:D, qsl],
                                start=True, stop=True)
                            nc.tensor.matmul(
                                sct[:, 512:1024], kdup[D:128, ksl],
                                qp[D:128, qsl], start=True, stop=True)
                            e_t = epool.tile([128, 1024], BF, tag="e",
                                             name=f"e{b}{pair}{qc}{kc}")
                            nc.scalar.activation(
                                e_t[:], sct[:], EXP, scale=float(SCALE))
                            va = vaug_b[b][kc]
                            nc.tensor.matmul(
                                avE[:], va[:], e_t[:, 0:512],
                                start=(kc == 0), stop=(kc == NKC - 1))
                            nc.tensor.matmul(
                                avO[:], va[:], e_t[:, 512:1024],
                                start=(kc == 0), stop=(kc == NKC - 1))
                        # --- evacuate PSUM first (frees av banks for the
                        # next pass), then the normalization chains ---
                        IDMASK = list(range(32))
                        zsE = small.tile([32, 512], F32, tag="zr", bufs=4,
                                         name=f"zrE{b}{pair}{qc}")
                        nc.vector.stream_shuffle(
                            zsE[0:32, :], avE[D:D + 32, :], IDMASK)
                        avc = small.tile([D, 512], F32, tag="avc",
                                         bufs=2, name=f"avc{b}{pair}{qc}")
                        nc.vector.tensor_copy(avc[:], avE[0:D, :])
                        zsO = small.tile([32, 512], F32, tag="zr", bufs=4,
                                         name=f"zrO{b}{pair}{qc}")
                        nc.vector.stream_shuffle(
                            zsO[0:32, :], avO[D:D + 32, :], IDMASK)
                        avs = small.tile([128, 512], F32, tag="avsb",
                                         name=f"avs{b}{pair}{qc}")
                        nc.vector.stream_shuffle(
                            avs[D:D + 32, :], avO[0:32, :], IDMASK)
                        nc.vector.stream_shuffle(
                            avs[D + 32:128, :], avO[32:D, :], IDMASK)
                        # normalization (off the PSUM critical path)
                        for head, zs, src_ap, rows in (
                            (0, zsE, avc[:], slice(0, D)),
                            (1, zsO, avs[D:128, :], slice(D, 128)),
                        ):
                            rt = small.tile([1, 512], F32, tag="rt", bufs=4,
                                            name=f"rt{b}{pair}{qc}{head}")
                            nc.vector.reciprocal_approx_fast(
                                rt[0:1, :], zs[0:1, :])
                            rb = small.tile([128, 512], F32, tag="rb", bufs=4,
                                            name=f"rb{b}{pair}{qc}{head}")
                            nc.gpsimd.partition_broadcast(rb[:], rt[0:1, :])
                            nc.vector.tensor_mul(
                                otu[rows, qsl],
                                src_ap, rb[rows, :])
                    if inline_oproj:
                        emit_oproj_qc(b, qc)

            def emit_oproj_qc(b, qc):
                for sc16 in range(4 * qc, 4 * qc + 4):
                    s128 = slice(sc16 * 128, (sc16 + 1) * 128)
                    for hcp in range(2):
                        opa = ps.tile([128, 512], F32, tag="pproj",
                                      name=f"opa{b}{sc16}{hcp}")
                        opb = ps.tile([128, 512], F32, tag="pproj",
                                      name=f"opb{b}{sc16}{hcp}")
                        hsl_a = slice((2 * hcp) * 512, (2 * hcp + 1) * 512)
                        hsl_b = slice((2 * hcp + 1) * 512, (2 * hcp + 2) * 512)
                        # group by stationary (otu chunk) to amortize LDW
                        nc.tensor.matmul(
                            opa[:], otu_b[b][0][:, s128], wo_t[0][:, hsl_a],
                            start=True, stop=False)
                        nc.tensor.matmul(
                            opb[:], otu_b[b][0][:, s128], wo_t[0][:, hsl_b],
                            start=True, stop=False)
                        nc.tensor.matmul(
                            opa[:], otu_b[b][1][:, s128], wo_t[1][:, hsl_a],
                            start=False, stop=True)
                        nc.tensor.matmul(
                            opb[:], otu_b[b][1][:, s128], wo_t[1][:, hsl_b],
                            start=False, stop=True)
                        for op, hsl in ((opa, hsl_a), (opb, hsl_b)):
                            ob = osb_pool.tile([128, 512], BF, tag="osb",
                                               name=f"ob{b}{sc16}{hsl.start}")
                            nc.vector.tensor_copy(ob[:], op[:])
                            nc.sync.dma_start(out=out[b, s128, hsl], in_=ob[:])

            def emit_oproj(b):
                for qc in range(NST):
                    emit_oproj_qc(b, qc)

            # emission order: P0 A0 P1 O0 A1 O1 — the tile scheduler fills
            # PE gaps of the ACT-bound attention with proj/o-proj matmuls.
            emit_proj(0)
            emit_attn(0)
            emit_proj(1)
            emit_oproj(0)
            emit_attn(1, inline_oproj=True)

    if not nc.is_finalized():
        nc.finalize()
    return nc


_NC = None
_RUN_KWARGS = {}
_LAST_RESULT = None


def _get_nc():
    global _NC
    if _NC is None:
        _NC = _build_nc()
    return _NC


def kernel(x, encoder_output, Wq, bq, Wk, bk, Wv, bv, Wo, bo):
    nc = _get_nc()
    xT = np.ascontiguousarray(np.asarray(x, np.float32).transpose(0, 2, 1)).astype(BF16)
    encT = np.ascontiguousarray(
        np.asarray(encoder_output, np.float32).transpose(0, 2, 1)).astype(BF16)
    Wq = np.asarray(Wq, np.float32)
    Wk = np.asarray(Wk, np.float32)
    Wv = np.asarray(Wv, np.float32)
    Wo = np.asarray(Wo, np.float32)
    in_maps = []
    for c in range(NCORES):
        csl = slice(c * CH, (c + 1) * CH)
        gsl = slice(c * D, (c + 1) * D)
        wkv_c = np.concatenate([Wk[:, gsl], Wv[:, gsl]], axis=1)
        in_maps.append({
            "xT": xT,
            "encT": encT,
            "wq": np.ascontiguousarray(Wq[:, csl]).astype(BF16),
            "wkv": np.ascontiguousarray(wkv_c).astype(BF16),
            "wo": np.ascontiguousarray(Wo[csl, :]).astype(BF16),
            "bq": np.ascontiguousarray(
                np.asarray(bq, np.float32)[csl].reshape(CH, 1)),
            "bk": np.ascontiguousarray(
                np.asarray(bk, np.float32)[gsl].reshape(D, 1)),
            "bv": np.ascontiguousarray(
                np.asarray(bv, np.float32)[gsl].reshape(D, 1)),
        })
    res = run_bass_kernel_spmd(nc, in_maps, list(range(NCORES)), **_RUN_KWARGS)
    global _LAST_RESULT
    _LAST_RESULT = res
    total = np.zeros((B, S, HID), np.float32)
    for c in range(NCORES):
        total += res.results[c]["out"].astype(np.float32)
    return total + np.asarray(bo, np.float32)
